# revision 1
# baseline (speedup 1.0000x reference)
"""Trainium2 Bass kernel for nn_Model_1331439862418.

4-layer stacked tanh-RNN with ReLU+AvgPool1d(k=7,s=5) between layers, final FC.
Data-parallel: B=512 sharded over 8 cores (64 batch each); each core runs the
full sequential scan chain.

Per-core design (all layers pipelined at step granularity):
  - layer-1 xproj: K=1 outer-product matmul from a DMA-streamed flat x.T buffer
  - layer>=2 xproj: ReLU+avgpool+input-projection fused into 7 accumulating
    "tap" matmuls (W_ih.T/7 @ relu_ring_slot) into the step's PSUM tile
  - recurrence: one matmul W_hh.T @ h_prev accumulated into the same PSUM bank
  - activation: tanh(psum + per-partition bias) on ScalarE -> h ring
  - relu: tensor_scalar_max on VectorE -> r ring (feeds next layer's taps)
  - FC: 35 accumulating taps (fc_w.T/7 slices @ r4 slots) + bias add, at tail

kernel(**inputs) takes FULL unsharded inputs, returns FULL [512, 10] output.
"""

import numpy as np

import concourse.bass as bass  # noqa: F401  (bass types used via bacc/tile)
import concourse.mybir as mybir
import concourse.tile as tile
from concourse import bacc
from concourse.bass_utils import run_bass_kernel_spmd

F32 = mybir.dt.float32
F16 = mybir.dt.float16
AF = mybir.ActivationFunctionType

NCORES = 8
B = 64          # batch per core
POOL_K, POOL_S = 7, 5
HS = [16, 32, 64, 128]
IS = [1, 16, 32, 64]

XCH = 64        # x-stream chunk length (steps)
XSLOTS = 4      # x-stream ring slots
RH = 8          # h ring slots per layer
MARGIN = 2      # parent steps between window-complete and child step emission


def seq_lens(T0):
    T = [T0]
    for _ in range(3):
        T.append((T[-1] - POOL_K) // POOL_S + 1)
    W4 = (T[3] - POOL_K) // POOL_S + 1
    return T, W4


def build(T0):
    """Build + compile the per-core Bass program. Returns compiled nc."""
    T, W4 = seq_lens(T0)
    nc = bacc.Bacc("TRN2", target_bir_lowering=False, debug=False,
                   num_devices=NCORES, enable_asserts=False)

    xq_d = nc.dram_tensor("xq", [1, T0 * B], F16, kind="ExternalInput")
    wih_d = [nc.dram_tensor(f"wih{l}", [IS[l], HS[l]], F16, kind="ExternalInput")
             for l in range(4)]
    whh_d = [nc.dram_tensor(f"whh{l}", [HS[l], HS[l]], F16, kind="ExternalInput")
             for l in range(4)]
    b_d = [nc.dram_tensor(f"b{l}", [HS[l], 1], F32, kind="ExternalInput")
           for l in range(4)]
    fcw_d = nc.dram_tensor("fcw", [W4 * 128, 10], F16, kind="ExternalInput")
    fcb_d = nc.dram_tensor("fcb", [10, 1], F32, kind="ExternalInput")
    out_d = nc.dram_tensor("out", [10, B], F32, kind="ExternalOutput")

    RR = [32, 32, 32, T[3]]     # relu ring slots per layer (r4 holds all steps)

    with tile.TileContext(nc) as tc:
        with (
            tc.tile_pool(name="const", bufs=1) as constp,
            tc.tile_pool(name="ring", bufs=1) as ringp,
            tc.tile_pool(name="ps1", bufs=2, space="PSUM") as ps1,
            tc.tile_pool(name="ps2", bufs=2, space="PSUM") as ps2,
            tc.tile_pool(name="ps3", bufs=2, space="PSUM") as ps3,
            tc.tile_pool(name="ps4", bufs=2, space="PSUM") as ps4,
        ):
            psp = [ps1, ps2, ps3, ps4]

            wih, whh, bias = [], [], []
            for l in range(4):
                w1 = constp.tile([IS[l], HS[l]], F16, tag=f"wih{l}")
                nc.sync.dma_start(out=w1, in_=wih_d[l].ap())
                wih.append(w1)
                w2 = constp.tile([HS[l], HS[l]], F16, tag=f"whh{l}")
                nc.sync.dma_start(out=w2, in_=whh_d[l].ap())
                whh.append(w2)
                bb = constp.tile([HS[l], 1], F32, tag=f"b{l}")
                nc.sync.dma_start(out=bb, in_=b_d[l].ap())
                bias.append(bb)
            fc_sb = constp.tile([128, W4, 10], F16, tag="fcw")
            nc.sync.dma_start(out=fc_sb,
                              in_=fcw_d.ap().rearrange("(j p) o -> p j o", p=128))
            fcb_sb = constp.tile([10, 1], F32, tag="fcb")
            nc.sync.dma_start(out=fcb_sb, in_=fcb_d.ap())

            xq = ringp.tile([1, XSLOTS * XCH * B], F16, tag="xq")
            h = [ringp.tile([HS[l], RH * B], F16, tag=f"h{l}", name=f"h{l}") for l in range(4)]
            r = [ringp.tile([HS[l], RR[l] * B], F16, tag=f"r{l}", name=f"r{l}") for l in range(4)]

            nchunks = (T0 + XCH - 1) // XCH

            def emit_xq_dma(c):
                if c >= nchunks:
                    return
                n = min(XCH, T0 - c * XCH) * B
                base = (c % XSLOTS) * XCH * B
                nc.sync.dma_start(out=xq[0:1, base:base + n],
                                  in_=xq_d.ap()[0:1, c * XCH * B:c * XCH * B + n])

            pswin = [dict() for _ in range(4)]   # layer -> window j -> psum tile
            ready = [None, [], [], []]           # ready-to-emit child windows

            def emit_tap(l, j, k):
                s = POOL_S * j + k               # parent-layer step index
                if k == 0:
                    pswin[l][j] = psp[l].tile([HS[l], B], F32, tag=f"ps{l}", name=f"psw{l}")
                ps = pswin[l][j]
                slot = s % RR[l - 1]
                nc.tensor.matmul(
                    ps, lhsT=wih[l], rhs=r[l - 1][:, slot * B:(slot + 1) * B],
                    start=(k == 0), stop=(k == POOL_K - 1 and j == 0),
                    skip_group_check=True)

            def emit_step(l, t):
                if l == 0:
                    ps = psp[0].tile([HS[0], B], F32, tag="ps0", name="ps0t")
                    off = ((t // XCH) % XSLOTS) * XCH * B + (t % XCH) * B
                    nc.tensor.matmul(ps, lhsT=wih[0], rhs=xq[0:1, off:off + B],
                                     start=True, stop=(t == 0),
                                     skip_group_check=True)
                else:
                    ps = pswin[l].pop(t)
                if t > 0:
                    hp = (t - 1) % RH
                    nc.tensor.matmul(ps, lhsT=whh[l],
                                     rhs=h[l][:, hp * B:(hp + 1) * B],
                                     start=False, stop=True,
                                     skip_group_check=True)
                hc = t % RH
                nc.scalar.activation(out=h[l][:, hc * B:(hc + 1) * B], in_=ps,
                                     func=AF.Tanh, bias=bias[l][:, 0:1], scale=1.0)
                rs = t % RR[l]
                nc.vector.tensor_scalar_max(r[l][:, rs * B:(rs + 1) * B],
                                            h[l][:, hc * B:(hc + 1) * B], 0.0)
                after_step(l, t)

            def after_step(l, s):
                if l == 3:
                    return                       # FC handled at tail
                c = l + 1
                n_child = T[c]
                jlo = max(0, -(-(s - (POOL_K - 1)) // POOL_S))  # ceil((s-6)/5)
                jhi = min(n_child - 1, s // POOL_S)
                for j in range(jlo, jhi + 1):
                    emit_tap(c, j, s - POOL_S * j)
                    if s - POOL_S * j == POOL_K - 1:
                        ready[c].append(j)
                while ready[c] and POOL_S * ready[c][0] + POOL_K - 1 + MARGIN <= s:
                    emit_step(c, ready[c].pop(0))

            # ---- main pipeline ----
            for c in range(min(XSLOTS - 1, nchunks)):
                emit_xq_dma(c)
            for t in range(T0):
                if t % XCH == 0:
                    emit_xq_dma(t // XCH + XSLOTS - 1)
                emit_step(0, t)
            for l in (1, 2, 3):                  # tail flush
                while ready[l]:
                    emit_step(l, ready[l].pop(0))

            # ---- FC tail ----
            ps_fc = psp[0].tile([10, B], F32, tag="ps0", name="psfc")
            for j in range(W4):
                for k in range(POOL_K):
                    s = POOL_S * j + k
                    nc.tensor.matmul(ps_fc, lhsT=fc_sb[:, j, :],
                                     rhs=r[3][:, s * B:(s + 1) * B],
                                     start=(j == 0 and k == 0),
                                     stop=(j == W4 - 1 and k == POOL_K - 1),
                                     skip_group_check=True)
            out_sb = constp.tile([10, B], F32, tag="out_sb")
            nc.vector.tensor_scalar_add(out_sb, ps_fc, fcb_sb[:, 0:1])
            nc.sync.dma_start(out=out_d.ap(), in_=out_sb)

    nc.compile()
    return nc


def prep_in_maps(inputs, T0):
    """Host-side prep: shard x, transpose/scale weights. Returns per-core maps."""
    T, W4 = seq_lens(T0)
    f = lambda a: np.ascontiguousarray(np.asarray(a, dtype=np.float32))
    x = f(inputs["x"]).reshape(-1, T0)          # [512, T0]
    nb = x.shape[0] // B

    common = {}
    for l in range(4):
        wi = f(inputs[f"w_ih{l + 1}"])          # [H, I]
        wh = f(inputs[f"w_hh{l + 1}"])          # [H, H]
        bi = f(inputs[f"b_ih{l + 1}"]) + f(inputs[f"b_hh{l + 1}"])
        scale = 1.0 if l == 0 else (1.0 / POOL_K)
        common[f"wih{l}"] = np.ascontiguousarray((wi * scale).T).astype(np.float16)
        common[f"whh{l}"] = np.ascontiguousarray(wh.T).astype(np.float16)
        common[f"b{l}"] = np.ascontiguousarray(bi.reshape(-1, 1))  # [H, 1]
    common["fcw"] = np.ascontiguousarray((f(inputs["fc_w"]) / POOL_K).T).astype(np.float16)
    common["fcb"] = np.ascontiguousarray(f(inputs["fc_b"]).reshape(-1, 1))

    in_maps = []
    for c in range(nb):
        m = dict(common)
        xc = x[c * B:(c + 1) * B]               # [B, T0]
        m["xq"] = np.ascontiguousarray(xc.T).reshape(1, T0 * B).astype(np.float16)
        in_maps.append(m)
    return in_maps


_NC_CACHE = {}


def _install_ntff_hook():
    """Register the axon NTFF profile hook (the agent image's antenv lacks
    axon_hooks, so run_bass_kernel_spmd's trace path can't find it)."""
    import sys
    import types
    if "antenv.axon_hooks" in sys.modules:
        return
    mod = types.ModuleType("antenv.axon_hooks")
    mod._hook = None
    mod.set_axon_ntff_profile_hook = lambda h: setattr(mod, "_hook", h)
    mod.get_axon_ntff_profile_hook = lambda: mod._hook
    sys.modules["antenv.axon_hooks"] = mod
    try:
        import antenv
        antenv.axon_hooks = mod
    except ImportError:
        pass
    try:
        from trn_agent_boot.trn_boot import _ntff_profile_via_ctypes
        mod._hook = _ntff_profile_via_ctypes("/opt/axon/libaxon_pjrt.so")
    except Exception as e:  # degrade to no tracing
        print("ntff hook install failed:", e)


def run(inputs, T0=3437, core_ids=None, trace=False):
    if trace:
        _install_ntff_hook()
    if T0 not in _NC_CACHE:
        _NC_CACHE[T0] = build(T0)
    nc = _NC_CACHE[T0]
    in_maps = prep_in_maps(inputs, T0)
    if core_ids is None:
        core_ids = list(range(len(in_maps)))
    res = run_bass_kernel_spmd(nc, in_maps, core_ids=core_ids, trace=trace)
    out = np.concatenate([res.results[i]["out"].T for i in range(len(in_maps))],
                         axis=0).astype(np.float32)
    return out, res


def kernel(**inputs) -> np.ndarray:
    out, _ = run(inputs)
    return out



# revision 10
# speedup vs baseline: 5.4034x; 5.4034x over previous
"""Trainium2 Bass kernel for nn_Model_1331439862418.

4-layer stacked tanh-RNN with ReLU+AvgPool1d(k=7,s=5) between layers, final FC.
Data-parallel: B=512 sharded over 8 cores (64 batch each).

Chunk-parallel scan: the RNN dynamics are strongly contractive (weights
U(-1/sqrt(H), 1/sqrt(H))), so each layer's time axis is split into parallel
chunks that warm up from h=0 over W=16 steps.  Chunks are packed both on
partitions (block-diagonal W_hh) and on the matmul free dim:

  L1: 7 partition-chunks x 8 free-chunks (M=112, N=512), 83 serial links
  L2: 4 x 8 (M=128, N=512), 43 links
  L3: 2 x 8 (M=128, N=512), 28 links
  L4: serial (M=128, N=64), 27 links

Per link: aux matmul (input projection, off critical path) + recurrence
matmul accumulate + one Tanh ACTIVATE (bias fused) + one DVE relu.
AvgPool runs on DVE pool_avg over completed windows; layers are glued
through canonical DRAM layouts with strided DMA gathers.
"""

import numpy as np

import concourse.bass as bass  # noqa: F401
import concourse.mybir as mybir
import concourse.tile as tile
from concourse import bacc
from concourse.bass_utils import run_bass_kernel_spmd

F32 = mybir.dt.float32
F16 = mybir.dt.float16
AF = mybir.ActivationFunctionType

NCORES = 8
B = 64
W = 16                  # warmup links per chunk
POOL_K, POOL_S = 7, 5

# per recurrent layer: H, I, CP (partition chunks), CF (free chunks), nw
# (pool windows owned per chunk).  NL = 5*nw + 2 + W serial links.
L1 = dict(H=16, I=1,  CP=7, CF=8, nw=13)
L2 = dict(H=32, I=16, CP=4, CF=8, nw=5)
L3 = dict(H=64, I=32, CP=2, CF=8, nw=2)
T4 = 27                 # layer-4 raw steps (serial)
W4 = 5                  # layer-4 pooled steps (FC input)

NL = {1: 5 * L1['nw'] + 2 + W,   # 83
      2: 5 * L2['nw'] + 2 + W,   # 43
      3: 5 * L3['nw'] + 2 + W}   # 28

# canonical DRAM pooled buffers: col j lives at (FRONT + j)*64; FRONT=W pad.
P1_COLS = 824           # writes to 16+56*13=744; reads to 25*31+43=818
P2_COLS = 200           # writes to 16+32*5=176; reads to 10*15+28=178
P3_COLS = 48            # writes to 16+16*2=48;  reads to 16+27=43


def build(T0=3437):
    nc = bacc.Bacc("TRN2", target_bir_lowering=False, debug=False,
                   num_devices=NCORES, enable_asserts=False)

    nl1, nl2, nl3 = NL[1], NL[2], NL[3]
    xall_d = nc.dram_tensor("xall", [7, nl1 * 512], F16, kind="ExternalInput")
    wx1_d = nc.dram_tensor("wx1", [7, 112], F16, kind="ExternalInput")
    whh_d = {1: nc.dram_tensor("whh1", [112, 112], F16, kind="ExternalInput"),
             2: nc.dram_tensor("whh2", [128, 128], F16, kind="ExternalInput"),
             3: nc.dram_tensor("whh3", [128, 128], F16, kind="ExternalInput"),
             4: nc.dram_tensor("whh4", [128, 128], F16, kind="ExternalInput")}
    wi_d = {2: nc.dram_tensor("wi2", [64, 128], F16, kind="ExternalInput"),
            3: nc.dram_tensor("wi3", [64, 128], F16, kind="ExternalInput"),
            4: nc.dram_tensor("wi4", [64, 128], F16, kind="ExternalInput")}
    b_d = {1: nc.dram_tensor("b1", [112, 1], F32, kind="ExternalInput"),
           2: nc.dram_tensor("b2", [128, 1], F32, kind="ExternalInput"),
           3: nc.dram_tensor("b3", [128, 1], F32, kind="ExternalInput"),
           4: nc.dram_tensor("b4", [128, 1], F32, kind="ExternalInput")}
    fcw_d = nc.dram_tensor("fcw", [128, W4 * 10], F16, kind="ExternalInput")
    fcb_d = nc.dram_tensor("fcb", [10, 1], F32, kind="ExternalInput")
    out_d = nc.dram_tensor("out", [10, B], F32, kind="ExternalOutput")

    p1d = nc.dram_tensor("p1d", [16, P1_COLS * B], F16, kind="Internal")
    p2d = nc.dram_tensor("p2d", [32, P2_COLS * B], F16, kind="Internal")
    p3d = nc.dram_tensor("p3d", [64, P3_COLS * B], F16, kind="Internal")

    with tile.TileContext(nc) as tc:
        with (
            tc.tile_pool(name="const", bufs=1) as constp,
            tc.tile_pool(name="ps", bufs=4, space="PSUM") as psp,
        ):
            # ---- constants ----
            wx1 = constp.tile([7, 112], F16, tag="wx1")
            nc.sync.dma_start(out=wx1, in_=wx1_d.ap())
            whh, wi, bias = {}, {}, {}
            for l in (1, 2, 3, 4):
                shp = [112, 112] if l == 1 else [128, 128]
                whh[l] = constp.tile(shp, F16, tag=f"whh{l}", name=f"whh{l}")
                nc.sync.dma_start(out=whh[l], in_=whh_d[l].ap())
                bias[l] = constp.tile([shp[0], 1], F32, tag=f"b{l}", name=f"b{l}")
                nc.sync.dma_start(out=bias[l], in_=b_d[l].ap())
                if l >= 2:
                    wi[l] = constp.tile([64, 128], F16, tag=f"wi{l}", name=f"wi{l}")
                    nc.sync.dma_start(out=wi[l], in_=wi_d[l].ap())
            fcw = constp.tile([128, W4, 10], F16, tag="fcw")
            nc.sync.dma_start(
                out=fcw, in_=fcw_d.ap().rearrange("p (j o) -> p j o", o=10))
            fcb = constp.tile([10, 1], F32, tag="fcb")
            nc.sync.dma_start(out=fcb, in_=fcb_d.ap())

            # zero out canonical-buffer pad regions that get read
            with tc.tile_pool(name="zp", bufs=1) as zp:
                zt = zp.tile([64, 80 * B], F16, tag="zt")
                nc.vector.memset(zt, 0.0)
                nc.sync.dma_start(out=p1d.ap()[:, 0:16 * B],
                                  in_=zt[0:16, 0:16 * B])
                nc.sync.dma_start(out=p1d.ap()[:, 744 * B:P1_COLS * B],
                                  in_=zt[0:16, 0:(P1_COLS - 744) * B])
                nc.sync.dma_start(out=p2d.ap()[:, 0:16 * B],
                                  in_=zt[0:32, 0:16 * B])
                nc.sync.dma_start(out=p2d.ap()[:, 176 * B:P2_COLS * B],
                                  in_=zt[0:32, 0:(P2_COLS - 176) * B])

            def recurrent_phase(lidx, cfg, nl, aux_lhsT, aux_k, x_sb, pooled,
                                dram_out, h, r, xslots=None, xstage=None):
                """Emit one chunk-parallel layer phase.

                xslots: if set, x_sb is a ring of that many 512-col link slots
                and xstage(piece) DMAs 8-link pieces into it.
                """
                CP, CF, nw, H = cfg['CP'], cfg['CF'], cfg['nw'], cfg['H']
                M = CP * H
                rv = r.rearrange("p (l c b) -> p l c b", c=CF, b=B)
                pv = pooled.rearrange("p (jj c b) -> p jj c b", c=CF, b=B)
                if xstage is not None:
                    xstage(0)
                    xstage(1)
                next_jj = 0
                for l in range(nl):
                    if xstage is not None and l % 8 == 0 and (l + 16) % 8 == 0:
                        xstage((l + 16) // 8)
                    xc = (l % xslots) if xslots else l
                    ps = psp.tile([128, 512], F32, tag="ps", name=f"ps{lidx}_{l}")
                    nc.tensor.matmul(ps[0:M, :], lhsT=aux_lhsT,
                                     rhs=x_sb[0:aux_k, xc * 512:(xc + 1) * 512],
                                     start=True, stop=(l == 0),
                                     skip_group_check=True)
                    if l > 0:
                        nc.tensor.matmul(ps[0:M, :], lhsT=whh[lidx][0:M, 0:M],
                                         rhs=h[0:M, ((l - 1) % 2) * 512:
                                               ((l - 1) % 2 + 1) * 512],
                                         start=False, stop=True,
                                         skip_group_check=True)
                    hs = h[0:M, (l % 2) * 512:(l % 2 + 1) * 512]
                    nc.scalar.activation(out=hs, in_=ps[0:M, :], func=AF.Tanh,
                                         bias=bias[lidx][0:M, 0:1], scale=1.0)
                    # relu in 2-link pairs (h ring slots 0,1 are contiguous)
                    r_ready = -1
                    if l % 2 == 1:
                        nc.vector.tensor_scalar_max(
                            r[:, (l - 1) * 512:(l + 1) * 512],
                            h[0:M, 0:1024], 0.0)
                        r_ready = l
                    elif l == nl - 1:
                        nc.vector.tensor_scalar_max(
                            r[:, l * 512:(l + 1) * 512], hs, 0.0)
                        r_ready = l
                    # pooled[jj] = sum_k r[5*jj+W+k] in 2-window blocks
                    while (next_jj < nw and r_ready >= 0
                           and 5 * (min(next_jj + 1, nw - 1)) + W + 6 <= r_ready):
                        jj = next_jj
                        njj = min(2, nw - jj)
                        lo = 5 * jj + W
                        dst = pv[:, jj:jj + njj, :, :]
                        hi = 5 * (njj - 1) + 1
                        nc.vector.tensor_tensor(
                            out=dst, in0=rv[:, lo:lo + hi:5, :, :],
                            in1=rv[:, lo + 1:lo + 1 + hi:5, :, :],
                            op=mybir.AluOpType.add)
                        for k in range(2, 7):
                            nc.vector.tensor_tensor(
                                out=dst, in0=dst,
                                in1=rv[:, lo + k:lo + k + hi:5, :, :],
                                op=mybir.AluOpType.add)
                        next_jj += njj
                # chunk (cp=0, cf=0) has no warmup: redo its pooled cols
                fdst = pv[0:H, 0:nw, 0, :]
                fhi = 5 * (nw - 1) + 1
                nc.vector.tensor_tensor(
                    out=fdst, in0=rv[0:H, 0:fhi:5, 0, :],
                    in1=rv[0:H, 1:1 + fhi:5, 0, :], op=mybir.AluOpType.add)
                for k in range(2, 7):
                    nc.vector.tensor_tensor(
                        out=fdst, in0=fdst,
                        in1=rv[0:H, k:k + fhi:5, 0, :],
                        op=mybir.AluOpType.add)
                # pooled -> canonical DRAM (3-dim APs per chunk)
                dv = dram_out.ap().rearrange("p (j b) -> p j b", b=B)
                for cp in range(CP):
                    for cf in range(CF):
                        c = cp * CF + cf
                        nc.sync.dma_start(
                            out=dv[:, 16 + c * nw:16 + (c + 1) * nw, :],
                            in_=pv[H * cp:H * (cp + 1), :, cf, :])

            # ================= layer 1 =================
            XS = 24     # x ring slots (links)
            with tc.tile_pool(name="l1", bufs=1) as lp:
                xring = lp.tile([7, XS * 512], F16, tag="xring")

                def xstage(p):
                    if p * 8 >= nl1:
                        return
                    lo, hi = p * 8, min((p + 1) * 8, nl1)
                    nc.sync.dma_start(
                        out=xring[:, (lo % XS) * 512:((lo % XS) + hi - lo) * 512],
                        in_=xall_d.ap()[:, lo * 512:hi * 512])

                h1 = lp.tile([112, 2 * 512], F16, tag="h1")
                r1 = lp.tile([112, nl1 * 512], F16, tag="r1")
                pooled1 = lp.tile([112, L1['nw'] * 512], F16, tag="pooled1")
                recurrent_phase(1, L1, nl1, wx1[0:7, 0:112], 7, xring, pooled1,
                                p1d, h1, r1, xslots=XS, xstage=xstage)

            # ================= layer 2 =================
            with tc.tile_pool(name="l2", bufs=1) as lp:
                x2 = lp.tile([64, nl2 * 512], F16, tag="x2")
                p1v = p1d.ap().rearrange("p (j b) -> p j b", b=B)
                x2v = x2.rearrange("p (l c b) -> p l c b", c=8, b=B)
                for cp in range(L2['CP']):
                    for cf in range(8):
                        c2 = cp * 8 + cf
                        j0 = 25 * c2      # = FRONT + s2(c2) = 16 + 25*c2 - 16
                        nc.sync.dma_start(
                            out=x2v[16 * cp:16 * (cp + 1), :, cf, :],
                            in_=p1v[:, j0:j0 + nl2, :])
                # chunk 0 fixup: s=0 (exact start), src j0 = FRONT
                nc.sync.dma_start(out=x2v[0:16, :, 0, :],
                                  in_=p1v[:, 16:16 + nl2, :])
                h2 = lp.tile([128, 2 * 512], F16, tag="h2")
                r2 = lp.tile([128, nl2 * 512], F16, tag="r2")
                pooled2 = lp.tile([128, L2['nw'] * 512], F16, tag="pooled2")
                recurrent_phase(2, L2, nl2, wi[2][0:64, 0:128], 64, x2,
                                pooled2, p2d, h2, r2)

            # ================= layer 3 =================
            with tc.tile_pool(name="l3", bufs=1) as lp:
                x3 = lp.tile([64, nl3 * 512], F16, tag="x3")
                p2v = p2d.ap().rearrange("p (j b) -> p j b", b=B)
                x3v = x3.rearrange("p (l c b) -> p l c b", c=8, b=B)
                for cp in range(L3['CP']):
                    for cf in range(8):
                        c3 = cp * 8 + cf
                        j0 = 10 * c3      # FRONT + s3 = 16 + 10*c3 - 16
                        nc.sync.dma_start(
                            out=x3v[32 * cp:32 * (cp + 1), :, cf, :],
                            in_=p2v[:, j0:j0 + nl3, :])
                nc.sync.dma_start(out=x3v[0:32, :, 0, :],
                                  in_=p2v[:, 16:16 + nl3, :])
                h3 = lp.tile([128, 2 * 512], F16, tag="h3")
                r3 = lp.tile([128, nl3 * 512], F16, tag="r3")
                pooled3 = lp.tile([128, L3['nw'] * 512], F16, tag="pooled3")
                recurrent_phase(3, L3, nl3, wi[3][0:64, 0:128], 64, x3,
                                pooled3, p3d, h3, r3)

            # ================= layer 4 (serial, exact) + FC =================
            with tc.tile_pool(name="l4", bufs=1) as lp:
                x4 = lp.tile([64, T4 * B], F16, tag="x4")
                p3v = p3d.ap().rearrange("p (j b) -> p j b", b=B)
                x4v = x4.rearrange("p (t b) -> p t b", b=B)
                nc.sync.dma_start(out=x4v, in_=p3v[:, 16:16 + T4, :])
                h4 = lp.tile([128, 2 * B], F16, tag="h4")
                r4 = lp.tile([128, T4 * B], F16, tag="r4")
                pooled4 = lp.tile([128, W4 * B], F16, tag="pooled4")
                r4v = r4.rearrange("p (t b) -> p t b", b=B)
                for t in range(T4):
                    ps = psp.tile([128, 512], F32, tag="ps", name=f"ps4_{t}")
                    nc.tensor.matmul(ps[:, 0:B], lhsT=wi[4][0:64, 0:128],
                                     rhs=x4[0:64, t * B:(t + 1) * B],
                                     start=True, stop=(t == 0),
                                     skip_group_check=True)
                    if t > 0:
                        nc.tensor.matmul(ps[:, 0:B], lhsT=whh[4],
                                         rhs=h4[:, ((t - 1) % 2) * B:
                                                ((t - 1) % 2 + 1) * B],
                                         start=False, stop=True,
                                         skip_group_check=True)
                    hs = h4[:, (t % 2) * B:(t % 2 + 1) * B]
                    nc.scalar.activation(out=hs, in_=ps[:, 0:B], func=AF.Tanh,
                                         bias=bias[4][:, 0:1], scale=1.0)
                    nc.vector.tensor_scalar_max(r4[:, t * B:(t + 1) * B],
                                                hs, 0.0)

                # pooled4[jj] = sum_k r4[5*jj+k]; 1/7 folded into fcw
                p4v = pooled4.rearrange("p (j b) -> p j b", b=B)
                p4hi = 5 * (W4 - 1) + 1
                nc.vector.tensor_tensor(
                    out=p4v, in0=r4v[:, 0:p4hi:5, :],
                    in1=r4v[:, 1:1 + p4hi:5, :], op=mybir.AluOpType.add)
                for k in range(2, 7):
                    nc.vector.tensor_tensor(
                        out=p4v, in0=p4v, in1=r4v[:, k:k + p4hi:5, :],
                        op=mybir.AluOpType.add)

                # ---- FC ----
                ps_fc = psp.tile([128, 512], F32, tag="ps", name="psfc")
                for j in range(W4):
                    nc.tensor.matmul(ps_fc[0:10, 0:B], lhsT=fcw[:, j, :],
                                     rhs=pooled4[:, j * B:(j + 1) * B],
                                     start=(j == 0), stop=(j == W4 - 1),
                                     skip_group_check=True)
                out_sb = lp.tile([10, B], F32, tag="out_sb")
                nc.vector.tensor_scalar_add(out_sb, ps_fc[0:10, 0:B],
                                            fcb[:, 0:1])
                nc.sync.dma_start(out=out_d.ap(), in_=out_sb)

    nc.compile()
    return nc


def _blockdiag(m, k):
    H = m.shape[0]
    out = np.zeros((k * H, k * m.shape[1]), np.float32)
    for i in range(k):
        out[i * H:(i + 1) * H, i * m.shape[1]:(i + 1) * m.shape[1]] = m
    return out


def prep_in_maps(inputs, T0=3437):
    f = lambda a: np.asarray(a, dtype=np.float32)
    x = f(inputs["x"]).reshape(-1, T0)           # [512, T0]
    nb = x.shape[0] // B

    wih = [f(inputs[f"w_ih{i}"]) for i in range(1, 5)]
    whh = [f(inputs[f"w_hh{i}"]) for i in range(1, 5)]
    bb = [f(inputs[f"b_ih{i}"]) + f(inputs[f"b_hh{i}"]) for i in range(1, 5)]

    common = {}
    wx1 = np.zeros((7, 112), np.float32)
    for cp in range(7):
        wx1[cp, 16 * cp:16 * (cp + 1)] = wih[0][:, 0]
    common["wx1"] = wx1.astype(np.float16)
    common["whh1"] = _blockdiag(whh[0].T, 7).astype(np.float16)
    common["whh2"] = _blockdiag(whh[1].T, 4).astype(np.float16)
    common["whh3"] = _blockdiag(whh[2].T, 2).astype(np.float16)
    common["whh4"] = whh[3].T.astype(np.float16)
    common["wi2"] = (_blockdiag(wih[1].T, 4) / POOL_K).astype(np.float16)
    common["wi3"] = (_blockdiag(wih[2].T, 2) / POOL_K).astype(np.float16)
    common["wi4"] = (wih[3].T / POOL_K).astype(np.float16)
    common["b1"] = np.tile(bb[0], 7).reshape(-1, 1).astype(np.float32)
    common["b2"] = np.tile(bb[1], 4).reshape(-1, 1).astype(np.float32)
    common["b3"] = np.tile(bb[2], 2).reshape(-1, 1).astype(np.float32)
    common["b4"] = bb[3].reshape(-1, 1).astype(np.float32)
    # fc: input index = w4*128 + c -> [128, W4, 10] -> [128, W4*10]
    fcw = (f(inputs["fc_w"]).T / POOL_K).reshape(W4, 128, 10).transpose(1, 0, 2)
    common["fcw"] = np.ascontiguousarray(fcw.reshape(128, W4 * 10)).astype(np.float16)
    common["fcb"] = f(inputs["fc_b"]).reshape(-1, 1).astype(np.float32)

    # layer-1 x staging: x_all[cp, l*512 + cf*64 + b] = x[b, s1(cp*8+cf) + l]
    nl1, nw1 = NL[1], L1['nw']
    c_idx = np.arange(56)
    s1 = np.maximum(0, 5 * nw1 * c_idx - W)
    s1[0] = 0
    t_idx = s1[:, None] + np.arange(nl1)[None, :]          # [56, nl1]
    valid = t_idx < T0
    t_clip = np.minimum(t_idx, T0 - 1)

    in_maps = []
    for c in range(nb):
        xc = x[c * B:(c + 1) * B]                          # [B, T0]
        g = xc[:, t_clip] * valid[None, :, :]              # [B, 56, nl1]
        g = g.reshape(B, 7, 8, nl1).transpose(1, 3, 2, 0)  # [7, nl1, 8, B]
        m = dict(common)
        m["xall"] = np.ascontiguousarray(g.reshape(7, nl1 * 512)).astype(np.float16)
        in_maps.append(m)
    return in_maps


_NC_CACHE = {}


def _install_ntff_hook():
    import sys
    import types
    if "antenv.axon_hooks" in sys.modules:
        return
    mod = types.ModuleType("antenv.axon_hooks")
    mod._hook = None
    mod.set_axon_ntff_profile_hook = lambda h: setattr(mod, "_hook", h)
    mod.get_axon_ntff_profile_hook = lambda: mod._hook
    sys.modules["antenv.axon_hooks"] = mod
    try:
        import antenv
        antenv.axon_hooks = mod
    except ImportError:
        pass
    try:
        from trn_agent_boot.trn_boot import _ntff_profile_via_ctypes
        mod._hook = _ntff_profile_via_ctypes("/opt/axon/libaxon_pjrt.so")
    except Exception as e:
        print("ntff hook install failed:", e)


def run(inputs, T0=3437, core_ids=None, trace=False):
    if trace:
        _install_ntff_hook()
    if T0 not in _NC_CACHE:
        _NC_CACHE[T0] = build(T0)
    nc = _NC_CACHE[T0]
    in_maps = prep_in_maps(inputs, T0)
    if core_ids is None:
        core_ids = list(range(len(in_maps)))
    res = run_bass_kernel_spmd(nc, in_maps, core_ids=core_ids, trace=trace)
    out = np.concatenate([res.results[i]["out"].T for i in range(len(in_maps))],
                         axis=0).astype(np.float32)
    return out, res


def kernel(**inputs) -> np.ndarray:
    out, _ = run(inputs)
    return out


# revision 13
# speedup vs baseline: 6.1321x; 1.1349x over previous
"""Trainium2 Bass kernel for nn_Model_1331439862418.

4-layer stacked tanh-RNN with ReLU+AvgPool1d(k=7,s=5) between layers, final FC.
Data-parallel: B=512 sharded over 8 cores (64 batch each).

Chunk-parallel scan: the RNN dynamics are strongly contractive (weights
U(-1/sqrt(H), 1/sqrt(H))), so each layer's time axis is split into parallel
chunks that warm up from h=0 over W=16 steps.  Chunks are packed both on
partitions (block-diagonal W_hh) and on the matmul free dim:

  L1: 7 partition-chunks x 8 free-chunks (M=112, N=512), 83 serial links
  L2: 4 x 8 (M=128, N=512), 43 links
  L3: 2 x 8 (M=128, N=512), 28 links
  L4: serial (M=128, N=64), 27 links

Per link: aux matmul (input projection, off critical path) + recurrence
matmul accumulate + one Tanh ACTIVATE (bias fused) + one DVE relu.
AvgPool runs on DVE pool_avg over completed windows; layers are glued
through canonical DRAM layouts with strided DMA gathers.
"""

import numpy as np

import concourse.bass as bass  # noqa: F401
from concourse.ap import AP
import concourse.mybir as mybir
import concourse.tile as tile
from concourse import bacc
from concourse.bass_utils import run_bass_kernel_spmd

F32 = mybir.dt.float32
F16 = mybir.dt.float16
AF = mybir.ActivationFunctionType

NCORES = 8
B = 64
W = 16                  # warmup links per chunk
POOL_K, POOL_S = 7, 5

# per recurrent layer: H, I, CP (partition chunks), CF (free chunks), nw
# (pool windows owned per chunk).  NL = 5*nw + 2 + W serial links.
L1 = dict(H=16, I=1,  CP=7, CF=8, nw=13)
L2 = dict(H=32, I=16, CP=4, CF=8, nw=5)
L3 = dict(H=64, I=32, CP=2, CF=8, nw=2)
T4 = 27                 # layer-4 raw steps (serial)
W4 = 5                  # layer-4 pooled steps (FC input)

NL = {1: 5 * L1['nw'] + 2 + W,   # 83
      2: 5 * L2['nw'] + 2 + W,   # 43
      3: 5 * L3['nw'] + 2 + W}   # 28

# canonical DRAM pooled buffers: col j lives at (FRONT + j)*64; FRONT=W pad.
P1_COLS = 824           # writes to 16+56*13=744; reads to 25*31+43=818
P2_COLS = 200           # writes to 16+32*5=176; reads to 10*15+28=178
P3_COLS = 48            # writes to 16+16*2=48;  reads to 16+27=43


def build(T0=3437):
    nc = bacc.Bacc("TRN2", target_bir_lowering=False, debug=False,
                   num_devices=NCORES, enable_asserts=False)

    nl1, nl2, nl3 = NL[1], NL[2], NL[3]
    xall_d = nc.dram_tensor("xall", [7, nl1 * 512], F16, kind="ExternalInput")
    wx1_d = nc.dram_tensor("wx1", [7, 112], F16, kind="ExternalInput")
    whh_d = {1: nc.dram_tensor("whh1", [112, 112], F16, kind="ExternalInput"),
             2: nc.dram_tensor("whh2", [128, 128], F16, kind="ExternalInput"),
             3: nc.dram_tensor("whh3", [128, 128], F16, kind="ExternalInput"),
             4: nc.dram_tensor("whh4", [128, 128], F16, kind="ExternalInput")}
    wi_d = {2: nc.dram_tensor("wi2", [64, 128], F16, kind="ExternalInput"),
            3: nc.dram_tensor("wi3", [64, 128], F16, kind="ExternalInput"),
            4: nc.dram_tensor("wi4", [64, 128], F16, kind="ExternalInput")}
    b_d = {1: nc.dram_tensor("b1", [112, 1], F32, kind="ExternalInput"),
           2: nc.dram_tensor("b2", [128, 1], F32, kind="ExternalInput"),
           3: nc.dram_tensor("b3", [128, 1], F32, kind="ExternalInput"),
           4: nc.dram_tensor("b4", [128, 1], F32, kind="ExternalInput")}
    fcw_d = nc.dram_tensor("fcw", [128, W4 * 10], F16, kind="ExternalInput")
    fcb_d = nc.dram_tensor("fcb", [10, 1], F32, kind="ExternalInput")
    out_d = nc.dram_tensor("out", [10, B], F32, kind="ExternalOutput")

    p1d = nc.dram_tensor("p1d", [16, P1_COLS * B], F16, kind="Internal")
    p2d = nc.dram_tensor("p2d", [32, P2_COLS * B], F16, kind="Internal")
    p3d = nc.dram_tensor("p3d", [64, P3_COLS * B], F16, kind="Internal")

    with tile.TileContext(nc) as tc:
        with (
            tc.tile_pool(name="const", bufs=1) as constp,
            tc.tile_pool(name="ps", bufs=4, space="PSUM") as psp,
        ):
            # ---- constants: L1-critical on the sync queue, rest on the
            # idle gpsimd queue so layer-1 links start immediately ----
            wx1 = constp.tile([7, 112], F16, tag="wx1")
            nc.sync.dma_start(out=wx1, in_=wx1_d.ap())
            whh, wi, bias = {}, {}, {}
            for l in (1, 2, 3, 4):
                eng = nc.sync if l == 1 else nc.gpsimd
                shp = [112, 112] if l == 1 else [128, 128]
                whh[l] = constp.tile(shp, F16, tag=f"whh{l}", name=f"whh{l}")
                eng.dma_start(out=whh[l], in_=whh_d[l].ap())
                bias[l] = constp.tile([shp[0], 1], F32, tag=f"b{l}", name=f"b{l}")
                eng.dma_start(out=bias[l], in_=b_d[l].ap())
                if l >= 2:
                    wi[l] = constp.tile([64, 128], F16, tag=f"wi{l}", name=f"wi{l}")
                    eng.dma_start(out=wi[l], in_=wi_d[l].ap())
            fcw = constp.tile([128, W4, 10], F16, tag="fcw")
            nc.gpsimd.dma_start(
                out=fcw, in_=fcw_d.ap().rearrange("p (j o) -> p j o", o=10))
            fcb = constp.tile([10, 1], F32, tag="fcb")
            nc.gpsimd.dma_start(out=fcb, in_=fcb_d.ap())

            # zero out canonical-buffer pad regions that get read
            with tc.tile_pool(name="zp", bufs=1) as zp:
                zt = zp.tile([64, 80 * B], F16, tag="zt")
                nc.gpsimd.memset(zt, 0.0)
                nc.gpsimd.dma_start(out=p1d.ap()[:, 0:16 * B],
                                    in_=zt[0:16, 0:16 * B])
                nc.gpsimd.dma_start(out=p1d.ap()[:, 744 * B:P1_COLS * B],
                                    in_=zt[0:16, 0:(P1_COLS - 744) * B])
                nc.gpsimd.dma_start(out=p2d.ap()[:, 0:16 * B],
                                    in_=zt[0:32, 0:16 * B])
                nc.gpsimd.dma_start(out=p2d.ap()[:, 176 * B:P2_COLS * B],
                                    in_=zt[0:32, 0:(P2_COLS - 176) * B])

            def recurrent_phase(lidx, cfg, nl, aux_lhsT, aux_k, aux_rhs,
                                pooled, dram_out, h, r, prefetch=None):
                """Emit one chunk-parallel layer phase.

                r/pooled are cf-major: col = cf*(links*64) + link*64 + b.
                aux_rhs(l) returns the aux matmul rhs AP for link l.
                """
                CP, CF, nw, H = cfg['CP'], cfg['CF'], cfg['nw'], cfg['H']
                M = CP * H
                rcv = r.rearrange("p (c l b) -> p c l b", l=nl, b=B)
                pcv = pooled.rearrange("p (c j b) -> p c j b", j=nw, b=B)
                hv = h.rearrange("p (s c b) -> p c s b", s=2, b=B)
                next_jj = 0
                for l in range(nl):
                    if prefetch is not None:
                        prefetch(l)
                    ps = psp.tile([128, 512], F32, tag="ps", name=f"ps{lidx}_{l}")
                    nc.tensor.matmul(ps[0:M, :], lhsT=aux_lhsT,
                                     rhs=aux_rhs(l),
                                     start=True, stop=(l == 0),
                                     skip_group_check=True)
                    if l > 0:
                        hprev = h[0:M, ((l - 1) % 2) * 512:
                                  ((l - 1) % 2 + 1) * 512]
                        nc.tensor.matmul(ps[0:M, :], lhsT=whh[lidx][0:M, 0:M],
                                         rhs=hprev, start=False, stop=True,
                                         skip_group_check=True)
                        # keep the PE HAM clock-gate warm during the ACT wait
                        wp = psp.tile([128, 512], F32, tag="pswarm",
                                      name=f"warm{lidx}_{l}")
                        nc.tensor.matmul(wp[0:M, :], lhsT=whh[lidx][0:M, 0:M],
                                         rhs=hprev, start=True, stop=True,
                                         skip_group_check=True)
                    hs = h[0:M, (l % 2) * 512:(l % 2 + 1) * 512]
                    nc.scalar.activation(out=hs, in_=ps[0:M, :], func=AF.Tanh,
                                         bias=bias[lidx][0:M, 0:1], scale=1.0)
                    # relu in 2-link pairs into cf-major r
                    r_ready = -1
                    if l % 2 == 1:
                        nc.vector.tensor_scalar_max(
                            rcv[:, :, l - 1:l + 1, :], hv[0:M], 0.0)
                        r_ready = l
                    elif l == nl - 1:
                        nc.vector.tensor_scalar_max(
                            rcv[:, :, l:l + 1, :], hv[0:M, :, l % 2:l % 2 + 1, :],
                            0.0)
                        r_ready = l
                    # pooled[jj] = sum_k r[5*jj+W+k] in 2-window blocks
                    while (next_jj < nw and r_ready >= 0
                           and 5 * (min(next_jj + 1, nw - 1)) + W + 6 <= r_ready):
                        jj = next_jj
                        njj = min(2, nw - jj)
                        lo = 5 * jj + W
                        hi = 5 * (njj - 1) + 1
                        dst = pcv[:, :, jj:jj + njj, :]
                        nc.vector.tensor_tensor(
                            out=dst, in0=rcv[:, :, lo:lo + hi:5, :],
                            in1=rcv[:, :, lo + 1:lo + 1 + hi:5, :],
                            op=mybir.AluOpType.add)
                        for k in range(2, 7):
                            nc.vector.tensor_tensor(
                                out=dst, in0=dst,
                                in1=rcv[:, :, lo + k:lo + k + hi:5, :],
                                op=mybir.AluOpType.add)
                        next_jj += njj
                # chunk (cp=0, cf=0) has no warmup: redo its pooled cols
                fdst = pcv[0:H, 0, :, :]
                fhi = 5 * (nw - 1) + 1
                nc.vector.tensor_tensor(
                    out=fdst, in0=rcv[0:H, 0, 0:fhi:5, :],
                    in1=rcv[0:H, 0, 1:1 + fhi:5, :], op=mybir.AluOpType.add)
                for k in range(2, 7):
                    nc.vector.tensor_tensor(
                        out=fdst, in0=fdst,
                        in1=rcv[0:H, 0, k:k + fhi:5, :],
                        op=mybir.AluOpType.add)
                # pooled -> canonical DRAM: one contiguous DMA per cp
                dv = dram_out.ap()
                for cp in range(CP):
                    nc.sync.dma_start(
                        out=dv[:, (16 + cp * CF * nw) * B:
                               (16 + (cp + 1) * CF * nw) * B],
                        in_=pooled[H * cp:H * (cp + 1), :])

            # ================= layer 1 =================
            XS = 24     # x ring slots (links)
            with tc.tile_pool(name="l1", bufs=1) as lp:
                xring = lp.tile([7, XS * 512], F16, tag="xring")

                def xstage(p):
                    if p * 8 >= nl1:
                        return
                    lo, hi = p * 8, min((p + 1) * 8, nl1)
                    nc.sync.dma_start(
                        out=xring[:, (lo % XS) * 512:((lo % XS) + hi - lo) * 512],
                        in_=xall_d.ap()[:, lo * 512:hi * 512])

                xstage(0)
                xstage(1)

                def prefetch1(l):
                    if l % 8 == 0:
                        xstage((l + 16) // 8)

                def aux_rhs1(l):
                    xc = l % XS
                    return xring[0:7, xc * 512:(xc + 1) * 512]

                h1 = lp.tile([112, 2 * 512], F16, tag="h1")
                r1 = lp.tile([112, nl1 * 512], F16, tag="r1")
                pooled1 = lp.tile([112, L1['nw'] * 512], F16, tag="pooled1")
                recurrent_phase(1, L1, nl1, wx1[0:7, 0:112], 7, aux_rhs1,
                                pooled1, p1d, h1, r1, prefetch=prefetch1)

            # ================= layer 2 =================
            with tc.tile_pool(name="l2", bufs=1) as lp:
                x2 = lp.tile([64, 8 * nl2 * B], F16, tag="x2")  # cf-major
                for cp in range(L2['CP']):
                    nc.sync.dma_start(
                        out=x2[16 * cp:16 * (cp + 1), :],
                        in_=AP(p1d.ap().tensor, 25 * 8 * cp * B,
                               [[P1_COLS * B, 16], [25 * B, 8], [1, nl2 * B]]))
                # chunk 0 fixup: s=0 (exact start), canonical col FRONT
                nc.sync.dma_start(out=x2[0:16, 0:nl2 * B],
                                  in_=p1d.ap()[:, 16 * B:(16 + nl2) * B])
                x2v = x2.rearrange("p (c l b) -> p c l b", l=nl2, b=B)

                def aux_rhs2(l):
                    return x2v[0:64, :, l, :]

                h2 = lp.tile([128, 2 * 512], F16, tag="h2")
                r2 = lp.tile([128, nl2 * 512], F16, tag="r2")
                pooled2 = lp.tile([128, L2['nw'] * 512], F16, tag="pooled2")
                recurrent_phase(2, L2, nl2, wi[2][0:64, 0:128], 64, aux_rhs2,
                                pooled2, p2d, h2, r2)

            # ================= layer 3 =================
            with tc.tile_pool(name="l3", bufs=1) as lp:
                x3 = lp.tile([64, 8 * nl3 * B], F16, tag="x3")  # cf-major
                for cp in range(L3['CP']):
                    nc.sync.dma_start(
                        out=x3[32 * cp:32 * (cp + 1), :],
                        in_=AP(p2d.ap().tensor, 10 * 8 * cp * B,
                               [[P2_COLS * B, 32], [10 * B, 8], [1, nl3 * B]]))
                nc.sync.dma_start(out=x3[0:32, 0:nl3 * B],
                                  in_=p2d.ap()[:, 16 * B:(16 + nl3) * B])
                x3v = x3.rearrange("p (c l b) -> p c l b", l=nl3, b=B)

                def aux_rhs3(l):
                    return x3v[0:64, :, l, :]

                h3 = lp.tile([128, 2 * 512], F16, tag="h3")
                r3 = lp.tile([128, nl3 * 512], F16, tag="r3")
                pooled3 = lp.tile([128, L3['nw'] * 512], F16, tag="pooled3")
                recurrent_phase(3, L3, nl3, wi[3][0:64, 0:128], 64, aux_rhs3,
                                pooled3, p3d, h3, r3)

            # ================= layer 4 (serial, exact) + FC =================
            with tc.tile_pool(name="l4", bufs=1) as lp:
                x4 = lp.tile([64, T4 * B], F16, tag="x4")
                p3v = p3d.ap().rearrange("p (j b) -> p j b", b=B)
                x4v = x4.rearrange("p (t b) -> p t b", b=B)
                nc.sync.dma_start(out=x4v, in_=p3v[:, 16:16 + T4, :])
                h4 = lp.tile([128, 2 * B], F16, tag="h4")
                r4 = lp.tile([128, T4 * B], F16, tag="r4")
                pooled4 = lp.tile([128, W4 * B], F16, tag="pooled4")
                r4v = r4.rearrange("p (t b) -> p t b", b=B)
                for t in range(T4):
                    ps = psp.tile([128, 512], F32, tag="ps", name=f"ps4_{t}")
                    nc.tensor.matmul(ps[:, 0:B], lhsT=wi[4][0:64, 0:128],
                                     rhs=x4[0:64, t * B:(t + 1) * B],
                                     start=True, stop=(t == 0),
                                     skip_group_check=True)
                    if t > 0:
                        nc.tensor.matmul(ps[:, 0:B], lhsT=whh[4],
                                         rhs=h4[:, ((t - 1) % 2) * B:
                                                ((t - 1) % 2 + 1) * B],
                                         start=False, stop=True,
                                         skip_group_check=True)
                    hs = h4[:, (t % 2) * B:(t % 2 + 1) * B]
                    nc.scalar.activation(out=hs, in_=ps[:, 0:B], func=AF.Tanh,
                                         bias=bias[4][:, 0:1], scale=1.0)
                    nc.vector.tensor_scalar_max(r4[:, t * B:(t + 1) * B],
                                                hs, 0.0)

                # pooled4[jj] = sum_k r4[5*jj+k]; 1/7 folded into fcw
                p4v = pooled4.rearrange("p (j b) -> p j b", b=B)
                p4hi = 5 * (W4 - 1) + 1
                nc.vector.tensor_tensor(
                    out=p4v, in0=r4v[:, 0:p4hi:5, :],
                    in1=r4v[:, 1:1 + p4hi:5, :], op=mybir.AluOpType.add)
                for k in range(2, 7):
                    nc.vector.tensor_tensor(
                        out=p4v, in0=p4v, in1=r4v[:, k:k + p4hi:5, :],
                        op=mybir.AluOpType.add)

                # ---- FC ----
                ps_fc = psp.tile([128, 512], F32, tag="ps", name="psfc")
                for j in range(W4):
                    nc.tensor.matmul(ps_fc[0:10, 0:B], lhsT=fcw[:, j, :],
                                     rhs=pooled4[:, j * B:(j + 1) * B],
                                     start=(j == 0), stop=(j == W4 - 1),
                                     skip_group_check=True)
                out_sb = lp.tile([10, B], F32, tag="out_sb")
                nc.vector.tensor_scalar_add(out_sb, ps_fc[0:10, 0:B],
                                            fcb[:, 0:1])
                nc.sync.dma_start(out=out_d.ap(), in_=out_sb)

    nc.compile()
    return nc


def _blockdiag(m, k):
    H = m.shape[0]
    out = np.zeros((k * H, k * m.shape[1]), np.float32)
    for i in range(k):
        out[i * H:(i + 1) * H, i * m.shape[1]:(i + 1) * m.shape[1]] = m
    return out


def prep_in_maps(inputs, T0=3437):
    f = lambda a: np.asarray(a, dtype=np.float32)
    x = f(inputs["x"]).reshape(-1, T0)           # [512, T0]
    nb = x.shape[0] // B

    wih = [f(inputs[f"w_ih{i}"]) for i in range(1, 5)]
    whh = [f(inputs[f"w_hh{i}"]) for i in range(1, 5)]
    bb = [f(inputs[f"b_ih{i}"]) + f(inputs[f"b_hh{i}"]) for i in range(1, 5)]

    common = {}
    wx1 = np.zeros((7, 112), np.float32)
    for cp in range(7):
        wx1[cp, 16 * cp:16 * (cp + 1)] = wih[0][:, 0]
    common["wx1"] = wx1.astype(np.float16)
    common["whh1"] = _blockdiag(whh[0].T, 7).astype(np.float16)
    common["whh2"] = _blockdiag(whh[1].T, 4).astype(np.float16)
    common["whh3"] = _blockdiag(whh[2].T, 2).astype(np.float16)
    common["whh4"] = whh[3].T.astype(np.float16)
    common["wi2"] = (_blockdiag(wih[1].T, 4) / POOL_K).astype(np.float16)
    common["wi3"] = (_blockdiag(wih[2].T, 2) / POOL_K).astype(np.float16)
    common["wi4"] = (wih[3].T / POOL_K).astype(np.float16)
    common["b1"] = np.tile(bb[0], 7).reshape(-1, 1).astype(np.float32)
    common["b2"] = np.tile(bb[1], 4).reshape(-1, 1).astype(np.float32)
    common["b3"] = np.tile(bb[2], 2).reshape(-1, 1).astype(np.float32)
    common["b4"] = bb[3].reshape(-1, 1).astype(np.float32)
    # fc: input index = w4*128 + c -> [128, W4, 10] -> [128, W4*10]
    fcw = (f(inputs["fc_w"]).T / POOL_K).reshape(W4, 128, 10).transpose(1, 0, 2)
    common["fcw"] = np.ascontiguousarray(fcw.reshape(128, W4 * 10)).astype(np.float16)
    common["fcb"] = f(inputs["fc_b"]).reshape(-1, 1).astype(np.float32)

    # layer-1 x staging: x_all[cp, l*512 + cf*64 + b] = x[b, s1(cp*8+cf) + l]
    nl1, nw1 = NL[1], L1['nw']
    c_idx = np.arange(56)
    s1 = np.maximum(0, 5 * nw1 * c_idx - W)
    s1[0] = 0
    t_idx = s1[:, None] + np.arange(nl1)[None, :]          # [56, nl1]
    valid = t_idx < T0
    t_clip = np.minimum(t_idx, T0 - 1)

    in_maps = []
    for c in range(nb):
        xc = x[c * B:(c + 1) * B]                          # [B, T0]
        g = xc[:, t_clip] * valid[None, :, :]              # [B, 56, nl1]
        g = g.reshape(B, 7, 8, nl1).transpose(1, 3, 2, 0)  # [7, nl1, 8, B]
        m = dict(common)
        m["xall"] = np.ascontiguousarray(g.reshape(7, nl1 * 512)).astype(np.float16)
        in_maps.append(m)
    return in_maps


_NC_CACHE = {}


def _install_ntff_hook():
    import sys
    import types
    if "antenv.axon_hooks" in sys.modules:
        return
    mod = types.ModuleType("antenv.axon_hooks")
    mod._hook = None
    mod.set_axon_ntff_profile_hook = lambda h: setattr(mod, "_hook", h)
    mod.get_axon_ntff_profile_hook = lambda: mod._hook
    sys.modules["antenv.axon_hooks"] = mod
    try:
        import antenv
        antenv.axon_hooks = mod
    except ImportError:
        pass
    try:
        from trn_agent_boot.trn_boot import _ntff_profile_via_ctypes
        mod._hook = _ntff_profile_via_ctypes("/opt/axon/libaxon_pjrt.so")
    except Exception as e:
        print("ntff hook install failed:", e)


def run(inputs, T0=3437, core_ids=None, trace=False):
    if trace:
        _install_ntff_hook()
    if T0 not in _NC_CACHE:
        _NC_CACHE[T0] = build(T0)
    nc = _NC_CACHE[T0]
    in_maps = prep_in_maps(inputs, T0)
    if core_ids is None:
        core_ids = list(range(len(in_maps)))
    res = run_bass_kernel_spmd(nc, in_maps, core_ids=core_ids, trace=trace)
    out = np.concatenate([res.results[i]["out"].T for i in range(len(in_maps))],
                         axis=0).astype(np.float32)
    return out, res


def kernel(**inputs) -> np.ndarray:
    out, _ = run(inputs)
    return out
